# revision 1
# baseline (speedup 1.0000x reference)
"""Trainium2 Bass kernel for nn_BatchedDynamicThresholdLIF.

Reference semantics (fp32), as lowered by the XLA/neuron pipeline
(divide-by-constant becomes multiply-by-reciprocal; verified against the
tensorizer IR of the compiled reference):
    t1 = (-65) - v ; t2 = t1 * 0.05f ; a = v + t2 ; vp = a + x_t
    s  = f32(vp >= th)
    th' = (th + select(s,5,0)) - ((th + 50) * 0.01f)
    v'  = select(s, -65, vp)
over T=1000 sequential steps on state (B=64, N=1024). Every op here
replicates that rounding exactly, so spike decisions are bit-exact.

Sharding: data-parallel over B across 8 cores (8 batch rows per core =
8192 state elements, laid out [128 partitions x 64 free]). The T
recurrence stays local per core; no cross-core communication.

Measured variants: this DVE/Pool split ran 1.514 ms HW (bit-exact);
all-DVE membrane (2.37 ms) and ACT-offload (2.0 ms) were slower.

Restructured recurrence (for engine balance): carry A(t) = fl(v(t) + t2):
    non-spiked: A = fl(vp - g),  g = fl(fl(vp+65) * 0.05f)
    spiked:     A = -65                  (v=-65 -> t2 = +0 exactly)
g/a_raw are computed from pre-reset vp on Pool (parallel to the compare
on DVE); the reset is applied afterwards with copy_predicated.
"""
import numpy as np

T, B, N = 1000, 64, 1024
NCORES = 8
BS = B // NCORES            # batch rows per core
S = BS * N                  # 8192 state elements per core
P = 128                     # SBUF partitions
F = S // P                  # 64 free elements per partition
KB = 50                     # timesteps per DMA block
R20 = 0.05                  # fp32(1/20)
R100 = 0.01                 # fp32(1/100)

_nc_cache = {}


def _build():
    import concourse.bacc as bacc
    import concourse.mybir as mybir
    import concourse.tile as tile

    f32 = mybir.dt.float32
    A = mybir.AluOpType
    nc = bacc.Bacc(None)
    x = nc.dram_tensor("x", [T, S], f32, kind="ExternalInput")
    so = nc.dram_tensor("s", [T, S], f32, kind="ExternalOutput")
    xv = x.rearrange("t (p j) -> p t j", p=P)
    sv = so.rearrange("t (p j) -> p t j", p=P)
    nblk = T // KB

    with tile.TileContext(nc) as tc:
        with tc.tile_pool(name="st", bufs=1) as stp, \
             tc.tile_pool(name="scr", bufs=2) as scr, \
             tc.tile_pool(name="xp", bufs=3) as xp, \
             tc.tile_pool(name="sp", bufs=3) as sp:
            aA = stp.tile([P, F], f32, name="aA")
            aB = stp.tile([P, F], f32, name="aB")
            th = stp.tile([P, F], f32, name="th")
            neg65 = stp.tile([P, F], f32, name="neg65")
            nc.vector.memset(aA, -65.0)
            nc.vector.memset(neg65, -65.0)
            nc.gpsimd.memset(th, -50.0)
            t = 0
            for b in range(nblk):
                xb = xp.tile([P, KB, F], f32, name="xb", tag="xb")
                nc.sync.dma_start(out=xb, in_=xv[:, b * KB:(b + 1) * KB, :])
                sb = sp.tile([P, KB, F], f32, name="sb", tag="sb")
                for k in range(KB):
                    a_cur, a_nxt = (aA, aB) if t % 2 == 0 else (aB, aA)
                    xt = xb[:, k, :]
                    st_ = sb[:, k, :]
                    vp = scr.tile([P, F], f32, name="vp", tag="vp")
                    g = scr.tile([P, F], f32, name="g", tag="g")
                    u2 = scr.tile([P, F], f32, name="u2", tag="u2")
                    u4 = scr.tile([P, F], f32, name="u4", tag="u4")
                    nc.vector.tensor_tensor(vp, a_cur, xt, A.add)
                    nc.vector.tensor_tensor(st_, vp, th, A.is_ge)
                    nc.gpsimd.tensor_scalar(g, vp, 65.0, R20, A.add, A.mult)
                    nc.gpsimd.tensor_tensor(a_nxt, vp, g, A.subtract)
                    nc.vector.copy_predicated(
                        a_nxt, st_.bitcast(mybir.dt.uint32), neg65)
                    nc.vector.scalar_tensor_tensor(u2, st_, 5.0, th, A.mult, A.add)
                    nc.gpsimd.tensor_scalar(u4, th, 50.0, R100, A.add, A.mult)
                    nc.gpsimd.tensor_tensor(th, u2, u4, A.subtract)
                    t += 1
                nc.sync.dma_start(out=sv[:, b * KB:(b + 1) * KB, :], in_=sb)
    nc.compile()
    return nc


def _get_nc():
    if "nc" not in _nc_cache:
        _nc_cache["nc"] = _build()
    return _nc_cache["nc"]


def kernel(weighted_input: np.ndarray) -> np.ndarray:
    from concourse.bass_utils import run_bass_kernel_spmd

    x = np.ascontiguousarray(np.asarray(weighted_input, dtype=np.float32))
    assert x.shape == (T, B, N), x.shape
    nc = _get_nc()
    in_maps = []
    for c in range(NCORES):
        xc = np.ascontiguousarray(x[:, c * BS:(c + 1) * BS, :].reshape(T, S))
        in_maps.append({"x": xc})
    res = run_bass_kernel_spmd(nc, in_maps, core_ids=list(range(NCORES)))
    out = np.empty((T, B, N), np.float32)
    for c in range(NCORES):
        out[:, c * BS:(c + 1) * BS, :] = res.results[c]["s"].reshape(T, BS, N)
    return out


if __name__ == "__main__":
    x = np.random.default_rng(0).standard_normal((T, B, N)).astype(np.float32) * 3.0
    s = kernel(x)
    print("spike rate:", s.mean())

